# revision 1
# baseline (speedup 1.0000x reference)
"""BitwiseWavenet Trainium2 kernel: 8-core SPMD, sequence-parallel sharding.

Layout: 4 partition groups of 32 channels = the 4 batches; L split 8 ways
across cores, each core computing a halo-extended window of W=10238 samples.
All convs are PE matmuls with block-diagonal (per-group) weights at
float32r; per-layer zero margins in SBUF implement conv zero-padding.
The two global sequence edges (first/last 1024 cols) are recomputed exactly
on the host in numpy (the device window's bias-cascade pollutes them).
"""
import sys
if '/opt/trn_rl_repo' not in sys.path:
    sys.path.insert(0, '/opt/trn_rl_repo')
import numpy as np

B, L = 4, 65536
N_CORES = 8
L_CORE = L // N_CORES            # 8192
HALO_L, HALO_R = 1024, 1022
W = HALO_L + L_CORE + HALO_R     # 10238
MARGIN = 256
WBUF = W + 2 * MARGIN            # 10750
CH = 512
NCHUNK = (W + CH - 1) // CH      # 20
NFIN = L_CORE // CH              # 16
MM_DT = "float32r"               # matmul compute dtype
R_VIA_PE = True                  # r update via identity-matmul PSUM accumulate

_NC_CACHE = {}


def _build_nc():
    key = (MM_DT, R_VIA_PE)
    if key in _NC_CACHE:
        return _NC_CACHE[key]
    import concourse.bacc as bacc
    import concourse.mybir as mybir
    import concourse.tile as tile
    F32 = mybir.dt.float32
    MMD = getattr(mybir.dt, MM_DT)
    A = mybir.AluOpType
    AF = mybir.ActivationFunctionType

    nc = bacc.Bacc("TRN2", target_bir_lowering=False, debug=False,
                   num_devices=N_CORES)
    x_d = nc.dram_tensor("xw", [4, W], MMD, kind="ExternalInput").ap()
    fgw_d = nc.dram_tensor("fgw", [20, 128, 512], MMD, kind="ExternalInput").ap()
    rsw_d = nc.dram_tensor("rsw", [20, 128, 256], MMD, kind="ExternalInput").ap()
    bias_d = nc.dram_tensor("biasw", [20, 128, 4], F32, kind="ExternalInput").ap()
    ident_d = nc.dram_tensor("identw", [128, 128], MMD, kind="ExternalInput").ap()
    startw_d = nc.dram_tensor("startw", [4, 128], MMD, kind="ExternalInput").ap()
    startb_d = nc.dram_tensor("startb", [128, 1], F32, kind="ExternalInput").ap()
    c1w_d = nc.dram_tensor("c1w", [128, 1024], MMD, kind="ExternalInput").ap()
    b1w_d = nc.dram_tensor("b1w", [128, 2], F32, kind="ExternalInput").ap()
    c2w_d = nc.dram_tensor("c2w", [128, 512], MMD, kind="ExternalInput").ap()
    b2w_d = nc.dram_tensor("b2w", [128, 2], F32, kind="ExternalInput").ap()
    out_d = nc.dram_tensor("out", [4, 256, L_CORE], F32, kind="ExternalOutput").ap()

    def mmd(ap):
        return ap if ap.dtype == MMD else ap.bitcast(MMD)

    with tile.TileContext(nc) as tc:
        with tc.tile_pool(name="big", bufs=1) as big, \
             tc.tile_pool(name="wts", bufs=2) as wts, \
             tc.tile_pool(name="cnk", bufs=3) as cnk:
            rA = big.tile([128, WBUF], MMD, tag="rA")
            rB = big.tile([128, WBUF], MMD, tag="rB")
            skip = big.tile([128, W], F32, tag="skip")
            ident = big.tile([128, 128], MMD, tag="ident")
            startw = big.tile([4, 128], MMD, tag="startw")
            startb = big.tile([128, 1], F32, tag="startb")
            c1w = big.tile([128, 4 * 256], MMD, tag="c1w")
            b1w = big.tile([128, 2], F32, tag="b1w")
            c2w = big.tile([128, 512], MMD, tag="c2w")
            b2w = big.tile([128, 2], F32, tag="b2w")
            nc.sync.dma_start(ident[:, :], ident_d[:, :])
            nc.sync.dma_start(startw[:, :], startw_d[:, :])
            nc.sync.dma_start(startb[:, :], startb_d[:, :])
            nc.sync.dma_start(c1w[:, :], c1w_d[:, :])
            nc.sync.dma_start(b1w[:, :], b1w_d[:, :])
            nc.sync.dma_start(c2w[:, :], c2w_d[:, :])
            nc.sync.dma_start(b2w[:, :], b2w_d[:, :])
            nc.gpsimd.memset(rA[:, :].bitcast(F32), 0.0)
            nc.gpsimd.memset(rB[:, :].bitcast(F32), 0.0)
            nc.gpsimd.memset(skip[:, :], 0.0)

            with tc.tile_pool(name="psw", bufs=2, space="PSUM") as ps:
                for c in range(NCHUNK):
                    c0 = c * CH
                    n = min(CH, W - c0)
                    base = MARGIN + c0
                    xc = cnk.tile([4, CH], MMD, tag="xc")
                    nc.sync.dma_start(xc[:, :n], x_d[:, c0:c0 + n])
                    pt = ps.tile([128, CH], F32, tag="f")
                    nc.tensor.matmul(pt[:, :n], mmd(startw[:, :]),
                                     mmd(xc[:, :n]), start=True, stop=True)
                    nc.scalar.activation(rA[:, base:base + n], pt[:, :n],
                                         AF.Identity, bias=startb[:, 0:1])
                cur, nxt = rA, rB
                for l in range(20):
                    blk, i = divmod(l, 10)
                    d = 2 ** i
                    offL, offR = (1, 0) if i == 0 else (d // 2, d // 2)
                    fg = wts.tile([128, 512], MMD, tag="fg")
                    rs = wts.tile([128, 256], MMD, tag="rs")
                    bi = wts.tile([128, 4], F32, tag="bi")
                    nc.sync.dma_start(fg[:, :], fgw_d[l, :, :])
                    nc.sync.dma_start(rs[:, :], rsw_d[l, :, :])
                    nc.sync.dma_start(bi[:, :], bias_d[l, :, :])
                    for c in range(NCHUNK):
                        c0 = c * CH
                        n = min(CH, W - c0)
                        base = MARGIN + c0
                        rhsL = mmd(cur[:, base - offL:base - offL + n])
                        rhsR = mmd(cur[:, base + offR:base + offR + n])
                        fps = ps.tile([128, CH], F32, tag="f")
                        nc.tensor.matmul(fps[:, :n], mmd(fg[:, 0:128]), rhsL,
                                         start=True, stop=False)
                        nc.tensor.matmul(fps[:, :n], mmd(fg[:, 128:256]), rhsR,
                                         start=False, stop=True)
                        gps = ps.tile([128, CH], F32, tag="g")
                        nc.tensor.matmul(gps[:, :n], mmd(fg[:, 256:384]), rhsL,
                                         start=True, stop=False)
                        nc.tensor.matmul(gps[:, :n], mmd(fg[:, 384:512]), rhsR,
                                         start=False, stop=True)
                        fsb = cnk.tile([128, CH], F32, tag="fsb")
                        nc.scalar.activation(fsb[:, :n], fps[:, :n],
                                             AF.Identity, bias=bi[:, 0:1])
                        lo = cnk.tile([128, CH], MMD, tag="lo")
                        nc.vector.scalar_tensor_tensor(
                            lo[:, :n], gps[:, :n], bi[:, 1:2], fsb[:, :n],
                            op0=A.add, op1=A.mult)
                        if R_VIA_PE:
                            rps = ps.tile([128, CH], F32, tag="racc")
                            nc.tensor.matmul(rps[:, :n], mmd(ident[:, :]),
                                             mmd(cur[:, base:base + n]),
                                             start=True, stop=False)
                            nc.tensor.matmul(rps[:, :n], mmd(rs[:, 0:128]),
                                             mmd(lo[:, :n]), start=False, stop=True)
                            nc.scalar.activation(nxt[:, base:base + n], rps[:, :n],
                                                 AF.Identity, bias=bi[:, 2:3])
                        else:
                            rps = ps.tile([128, CH], F32, tag="racc")
                            nc.tensor.matmul(rps[:, :n], mmd(rs[:, 0:128]),
                                             mmd(lo[:, :n]), start=True, stop=True)
                            nc.vector.scalar_tensor_tensor(
                                nxt[:, base:base + n], rps[:, :n], bi[:, 2:3],
                                cur[:, base:base + n], op0=A.add, op1=A.add)
                        sps = ps.tile([128, CH], F32, tag="skip")
                        nc.tensor.matmul(sps[:, :n], mmd(rs[:, 128:256]),
                                         mmd(lo[:, :n]), start=True, stop=True)
                        nc.vector.scalar_tensor_tensor(
                            skip[:, c0:c0 + n], sps[:, :n], bi[:, 3:4],
                            skip[:, c0:c0 + n], op0=A.add, op1=A.add)
                    cur, nxt = nxt, cur

            with tc.tile_pool(name="psf", bufs=2, space="PSUM") as psf:
                for c in range(NFIN):
                    c0 = HALO_L + c * CH
                    rl = cnk.tile([128, CH], MMD, tag="rl")
                    nc.scalar.activation(rl[:, :], skip[:, c0:c0 + CH], AF.Relu)
                    for g in range(4):
                        o1sb = []
                        for h in range(2):
                            o1ps = psf.tile([128, CH], F32, tag=f"o1_{h}")
                            nc.tensor.matmul(
                                o1ps[:, :],
                                mmd(c1w[:, (2 * g + h) * 128:(2 * g + h) * 128 + 128]),
                                mmd(rl[:, :]), start=True, stop=True)
                            t = cnk.tile([128, CH], MMD, tag=f"o1sb_{h}")
                            nc.vector.tensor_scalar(t[:, :], o1ps[:, :],
                                                    b1w[:, h:h + 1], 0.0,
                                                    op0=A.add, op1=A.max)
                            o1sb.append(t)
                        for h2 in range(2):
                            o2ps = psf.tile([128, CH], F32, tag=f"o2_{h2}")
                            for h in range(2):
                                j = 2 * h + h2
                                nc.tensor.matmul(o2ps[:, :],
                                                 mmd(c2w[:, j * 128:(j + 1) * 128]),
                                                 mmd(o1sb[h][:, :]),
                                                 start=(h == 0), stop=(h == 1))
                            o2sb = cnk.tile([128, CH], F32, tag="o2sb")
                            nc.scalar.activation(o2sb[:, :], o2ps[:, :],
                                                 AF.Identity, bias=b2w[:, h2:h2 + 1])
                            nc.sync.dma_start(
                                out_d[g, 128 * h2:128 * (h2 + 1), c * CH:(c + 1) * CH],
                                o2sb[:, :])
    nc.compile()
    _NC_CACHE[key] = nc
    return nc


def _host_arrays(inputs):
    """Builds the shared (core-independent) weight arrays."""
    def make_bd(Wm):
        bd = np.zeros((128, 128), np.float32)
        for g in range(4):
            bd[32 * g:32 * g + 32, 32 * g:32 * g + 32] = Wm.T
        return bd

    fgw = np.zeros((20, 128, 512), np.float32)
    rsw = np.zeros((20, 128, 256), np.float32)
    biasw = np.zeros((20, 128, 4), np.float32)
    for l in range(20):
        blk, i = divmod(l, 10)
        fgw[l, :, 0:128] = make_bd(inputs['filt_w'][blk, i, :, :, 0])
        fgw[l, :, 128:256] = make_bd(inputs['filt_w'][blk, i, :, :, 1])
        fgw[l, :, 256:384] = make_bd(inputs['gate_w'][blk, i, :, :, 0])
        fgw[l, :, 384:512] = make_bd(inputs['gate_w'][blk, i, :, :, 1])
        rsw[l, :, 0:128] = make_bd(inputs['res_w'][blk, i, :, :, 0])
        rsw[l, :, 128:256] = make_bd(inputs['skip_w'][blk, i, :, :, 0])
        biasw[l, :, 0] = np.tile(inputs['filt_b'][blk, i], 4)
        biasw[l, :, 1] = np.tile(inputs['gate_b'][blk, i], 4)
        biasw[l, :, 2] = np.tile(inputs['res_b'][blk, i], 4)
        biasw[l, :, 3] = np.tile(inputs['skip_b'][blk, i], 4)
    identw = np.eye(128, dtype=np.float32)
    startw = np.zeros((4, 128), np.float32)
    for g in range(4):
        startw[g, 32 * g:32 * g + 32] = inputs['w_start'][:, 0, 0]
    startb = np.tile(inputs['b_start'], 4).reshape(128, 1).astype(np.float32)
    c1w = np.zeros((4, 128, 256), np.float32)
    for g in range(4):
        for h in range(2):
            c1w[g, 32 * g:32 * g + 32, 128 * h:128 * h + 128] = \
                inputs['w_end1'][128 * h:128 * h + 128, :, 0].T
    b1w = np.stack([inputs['b_end1'][0:128], inputs['b_end1'][128:256]],
                   axis=1).astype(np.float32)
    c2w = np.zeros((128, 512), np.float32)
    for h in range(2):
        for h2 in range(2):
            c2w[:, (2 * h + h2) * 128:(2 * h + h2) * 128 + 128] = \
                inputs['w_end2'][128 * h2:128 * h2 + 128, 128 * h:128 * h + 128, 0].T
    b2w = np.stack([inputs['b_end2'][0:128], inputs['b_end2'][128:256]],
                   axis=1).astype(np.float32)
    # flatten to SBUF layout: col block (2g+h) holds group-g/out-half-h weights
    c1w_sb = np.ascontiguousarray(
        c1w.transpose(1, 0, 2).reshape(128, 1024))
    return dict(fgw=fgw, rsw=rsw, biasw=biasw, identw=identw, startw=startw,
                startb=startb, c1w=c1w_sb, b1w=b1w, c2w=c2w, b2w=b2w)


def _np_reference_strip(inputs, x_strip):
    """Exact fp32 reference on a short strip (true zero-padded edges)."""
    S = x_strip.shape[1]

    def layer_conv(r, Wm, b, offL, offR):
        rp = np.pad(r, ((0, 0), (0, 0), (offL, offR)))
        return (np.einsum('oc,bct->bot', Wm[:, :, 0], rp[:, :, 0:S]) +
                np.einsum('oc,bct->bot', Wm[:, :, 1],
                          rp[:, :, offL + offR:offL + offR + S]) +
                b[None, :, None])

    r = (inputs['w_start'][:, 0, 0][None, :, None] * x_strip[:, None, :] +
         inputs['b_start'][None, :, None])
    skip_total = np.zeros_like(r)
    for blk in range(2):
        skip = np.zeros_like(r)
        for i in range(10):
            d = 2 ** i
            offL, offR = (1, 0) if i == 0 else (d // 2, d // 2)
            f = layer_conv(r, inputs['filt_w'][blk, i], inputs['filt_b'][blk, i], offL, offR)
            g = layer_conv(r, inputs['gate_w'][blk, i], inputs['gate_b'][blk, i], offL, offR)
            lo = f * g
            skip = skip + np.einsum('oc,bct->bot', inputs['skip_w'][blk, i][:, :, 0], lo) \
                + inputs['skip_b'][blk, i][None, :, None]
            r = r + np.einsum('oc,bct->bot', inputs['res_w'][blk, i][:, :, 0], lo) \
                + inputs['res_b'][blk, i][None, :, None]
        skip_total = skip_total + skip
    out = np.maximum(skip_total, 0)
    out = np.maximum(np.einsum('oc,bct->bot', inputs['w_end1'][:, :, 0], out) +
                     inputs['b_end1'][None, :, None], 0)
    return (np.einsum('oc,bct->bot', inputs['w_end2'][:, :, 0], out) +
            inputs['b_end2'][None, :, None])


def run(trace=False, **inputs):
    from concourse.bass_utils import run_bass_kernel_spmd
    inputs = {k: np.ascontiguousarray(np.asarray(v, np.float32)) for k, v in inputs.items()}
    nc = _build_nc()
    shared = _host_arrays(inputs)
    x = inputs['x']  # [4, 1, L]
    in_maps = []
    for core in range(N_CORES):
        s = core * L_CORE
        xw = np.zeros((4, W), np.float32)
        lo_g, hi_g = s - HALO_L, s + L_CORE + HALO_R
        lo_c, hi_c = max(lo_g, 0), min(hi_g, L)
        xw[:, lo_c - lo_g: lo_c - lo_g + (hi_c - lo_c)] = x[:, 0, lo_c:hi_c]
        m = {"xw": xw}
        m.update(shared)
        # rename keys to dram tensor names
        in_maps.append({"xw": xw, "fgw": shared['fgw'], "rsw": shared['rsw'],
                        "biasw": shared['biasw'], "identw": shared['identw'],
                        "startw": shared['startw'], "startb": shared['startb'],
                        "c1w": shared['c1w'], "b1w": shared['b1w'],
                        "c2w": shared['c2w'], "b2w": shared['b2w']})
    res = run_bass_kernel_spmd(nc, in_maps, core_ids=list(range(N_CORES)),
                               trace=trace)
    out = np.zeros((B, 256, L), np.float32)
    for core in range(N_CORES):
        out[:, :, core * L_CORE:(core + 1) * L_CORE] = res.results[core]["out"]
    # host edge fix (device window edges differ from true sequence edges)
    STRIP = 2048
    left = _np_reference_strip(inputs, x[:, 0, :STRIP])
    out[:, :, :HALO_L] = left[:, :, :HALO_L]
    right = _np_reference_strip(inputs, x[:, 0, L - STRIP:])
    out[:, :, L - HALO_L:] = right[:, :, STRIP - HALO_L:]
    return out, res


def kernel(**inputs) -> np.ndarray:
    out, _ = run(trace=False, **inputs)
    return out



# revision 2
# speedup vs baseline: 1.0781x; 1.0781x over previous
"""BitwiseWavenet TRN2 kernel v2: fp8-DoubleRow layer stack, bf16 head.

8-core SPMD, sequence-parallel (L/8 per core + 1024/1022 halo recompute).
Partition layout: 4 batch-groups x 32 channels = 128 partitions.

Layer loop in fp8 e4m3 (numpy-sim validated rel-err ~4.6e-3 vs 2e-2 gate):
  - residual r lives ONLY as scaled fp8 r~ = r*S_r; the "+ r" of the residual
    update is an identity K-tile (exact pow2 diagonal) inside the same
    DoubleRow matmul as Wr @ p~.
  - f/g dilated convs: ONE DoubleRow matmul pairs both taps via a strided
    rhs AP (any dilation - verified on hw down to stride 1).
  - a == 1: weight scales satisfy S_wf*S_wg = S_p/S_r^2 so no descale op.
  - biases: bg in the gsb activation bias, bf in the p~ stt scalar slot,
    br in the r~ activation bias, bs deferred entirely to the head relu.
  - skip = Ws @ p~ deferred per 4-layer quad: p~ kept in a 4-slot ring
    (window part) + scratch, 2 DoubleRows + 1 stt per quad per chunk.
Head (w_end1/w_end2) in bf16 (fp8 there fails the gate).
Scales: power-of-2, calibrated at runtime from a short numpy probe.
Sequence edges (first/last 1024 cols) recomputed exactly on host.
"""
import sys
if '/opt/trn_rl_repo' not in sys.path:
    sys.path.insert(0, '/opt/trn_rl_repo')
import dataclasses
import numpy as np
import ml_dtypes

B, L = 4, 65536
N_CORES = 8
L_CORE = L // N_CORES            # 8192
HALO_L, HALO_R = 1024, 1022
W = HALO_L + L_CORE + HALO_R     # 10238
MARGIN = 256
WBUF = 10752                     # MARGIN + W + tail pad (16-aligned)
CH = 512
NFIN = L_CORE // CH              # 16
NLAYERS = 20
NQUAD = 5

# fp8 mega-tile region offsets (elements).  Layout keeps every DoubleRow
# rhs region pair within the signed 16-bit stride field (|stride|<=32767):
# [ring0, ring2, scr0, r0, r1, scr1, ring1, ring3, ones]
RING = [0, 59392, 8192, 67584]   # ring slot base by (layer % 4)
SCR0 = 16384
R0 = 27136
R1 = 37888
SCR1 = 48640
ONES = 75776
MEGA_W = 76800

E4 = ml_dtypes.float8_e4m3
BF = ml_dtypes.bfloat16

_NC_CACHE = {}


def _q8(x):
    return np.asarray(x, np.float32).astype(E4).astype(np.float32)


def _pow2(maxval, target=120.0):
    if maxval <= 0:
        return 1.0
    return float(2.0 ** np.floor(np.log2(target / maxval)))


def _layer_geom():
    offs = []
    for l in range(NLAYERS):
        i = l % 10
        d = 2 ** i
        offs.append((1, 0) if i == 0 else (d // 2, d // 2))
    P = [None] * NLAYERS
    lo, hi = HALO_L, HALO_L + L_CORE
    for l in range(NLAYERS - 1, -1, -1):
        clo = (lo // CH) * CH
        chi = min(((hi + CH - 1) // CH) * CH, W)
        P[l] = (clo, chi)
        offL, offR = offs[l]
        lo, hi = max(clo - offL, 0), min(chi + offR, W)
    start_range = ((lo // CH) * CH, min(((hi + CH - 1) // CH) * CH, W))
    return offs, P, start_range


def _build_nc(cfg):
    key = tuple(sorted(cfg.items()))
    if key in _NC_CACHE:
        return _NC_CACHE[key]
    import concourse.bacc as bacc
    import concourse.mybir as mybir
    import concourse.tile as tile
    F32 = mybir.dt.float32
    F32R = mybir.dt.float32r
    BF16 = mybir.dt.bfloat16
    FP8 = mybir.dt.float8e4
    A = mybir.AluOpType
    AF = mybir.ActivationFunctionType
    DR = mybir.MatmulPerfMode.DoubleRow

    offs, P, start_range = _layer_geom()
    rinv_imm = cfg['rinv_imm']
    qinv_imm = cfg['qinv_imm']
    s_r = cfg['s_r']

    nc = bacc.Bacc("TRN2", target_bir_lowering=False, debug=False,
                   num_devices=N_CORES)
    x_d = nc.dram_tensor("xw", [4, WBUF], F32R, kind="ExternalInput").ap()
    fgw_d = nc.dram_tensor("fgw", [128, NLAYERS * 2 * 256], F32, kind="ExternalInput").ap()
    resw_d = nc.dram_tensor("resw", [128, NLAYERS * 256], F32, kind="ExternalInput").ap()
    skw_d = nc.dram_tensor("skw", [128, NQUAD * 2 * 256], F32, kind="ExternalInput").ap()
    bfc_d = nc.dram_tensor("bfc", [128, NLAYERS], F32, kind="ExternalInput").ap()
    bgc_d = nc.dram_tensor("bgc", [128, NLAYERS], F32, kind="ExternalInput").ap()
    brc_d = nc.dram_tensor("brc", [128, NLAYERS], F32, kind="ExternalInput").ap()
    bst_d = nc.dram_tensor("bst", [128, 1], F32, kind="ExternalInput").ap()
    startw_d = nc.dram_tensor("startw", [4, 128], F32R, kind="ExternalInput").ap()
    startb_d = nc.dram_tensor("startb", [128, 1], F32, kind="ExternalInput").ap()
    c1w_d = nc.dram_tensor("c1w", [128, 1024], F32, kind="ExternalInput").ap()
    b1w_d = nc.dram_tensor("b1w", [128, 2], F32, kind="ExternalInput").ap()
    c2w_d = nc.dram_tensor("c2w", [128, 512], F32, kind="ExternalInput").ap()
    b2w_d = nc.dram_tensor("b2w", [128, 2], F32, kind="ExternalInput").ap()
    out_d = nc.dram_tensor("out", [4, 256, L_CORE], F32, kind="ExternalOutput").ap()

    with tile.TileContext(nc) as tc:
        with tc.tile_pool(name="big", bufs=1) as big, \
             tc.tile_pool(name="cnk", bufs=3) as cnk:
            mega = big.tile([128, MEGA_W], FP8, tag="mega")
            fgw = big.tile([128, NLAYERS * 2, 256], FP8, tag="fgw")
            resw = big.tile([128, NLAYERS, 256], FP8, tag="resw")
            skw = big.tile([128, NQUAD * 2, 256], FP8, tag="skw")
            skip = big.tile([128, L_CORE], F32, tag="skip")
            bfc = big.tile([128, NLAYERS], F32, tag="bfc")
            bgc = big.tile([128, NLAYERS], F32, tag="bgc")
            brc = big.tile([128, NLAYERS], F32, tag="brc")
            bst = big.tile([128, 1], F32, tag="bst")
            startw = big.tile([4, 128], F32R, tag="startw")
            startb = big.tile([128, 1], F32, tag="startb")
            c1w = big.tile([128, 1024], BF16, tag="c1w")
            b1w = big.tile([128, 2], F32, tag="b1w")
            c2w = big.tile([128, 512], BF16, tag="c2w")
            b2w = big.tile([128, 2], F32, tag="b2w")

            nc.sync.dma_start(bfc[:, :], bfc_d[:, :])
            nc.sync.dma_start(bgc[:, :], bgc_d[:, :])
            nc.sync.dma_start(brc[:, :], brc_d[:, :])
            nc.sync.dma_start(bst[:, :], bst_d[:, :])
            nc.sync.dma_start(startw[:, :], startw_d[:, :])
            nc.sync.dma_start(startb[:, :], startb_d[:, :])
            nc.sync.dma_start(b1w[:, :], b1w_d[:, :])
            nc.sync.dma_start(b2w[:, :], b2w_d[:, :])
            nc.gpsimd.memset(mega[:, 0:ONES].bitcast(F32), 0.0)
            nc.gpsimd.memset(mega[:, ONES:MEGA_W], 1.0)
            nc.gpsimd.memset(skip[:, :], 0.0)

            with tc.tile_pool(name="stg", bufs=2) as stg:
                SW = 2560
                def conv_in(dst2d, src_d, total):
                    for o in range(0, total, SW):
                        nn = min(SW, total - o)
                        t = stg.tile([128, SW], F32, tag="st")
                        nc.sync.dma_start(t[:, :nn], src_d[:, o:o + nn])
                        nc.scalar.activation(dst2d[:, o:o + nn], t[:, :nn], AF.Identity)
                conv_in(fgw[:, :, :].rearrange("p a b -> p (a b)"), fgw_d, NLAYERS * 2 * 256)
                conv_in(resw[:, :, :].rearrange("p a b -> p (a b)"), resw_d, NLAYERS * 256)
                conv_in(skw[:, :, :].rearrange("p a b -> p (a b)"), skw_d, NQUAD * 2 * 256)
                conv_in(c1w, c1w_d, 1024)
                conv_in(c2w, c2w_d, 512)

            def w3(tile2d, idx):
                return tile2d[:, idx, :].rearrange("p (two m) -> p two m", two=2)

            def dr_rhs(off0, stride, n):
                base = mega[:, off0: off0 + n]
                return dataclasses.replace(
                    base, ap=[list(base.ap[0]), [int(stride), 2], [1, n]])

            def p_off(l, c0):
                """element offset of p~[layer l] at window col c0"""
                if HALO_L <= c0 < HALO_L + L_CORE:
                    return RING[l % 4] + (c0 - HALO_L)
                return (SCR0 if l % 2 == 0 else SCR1) + MARGIN + c0

            # ---- start conv ----
            with tc.tile_pool(name="ps0", bufs=2, space="PSUM") as ps0:
                s_lo, s_hi = start_range
                for c0 in range(s_lo, s_hi, CH):
                    n = min(CH, W - c0)
                    xc = cnk.tile([4, CH], F32R, tag="xc")
                    nc.sync.dma_start(xc[:, :n], x_d[:, MARGIN + c0: MARGIN + c0 + n])
                    pt = ps0.tile([128, CH], F32, tag="s")
                    nc.tensor.matmul(pt[:, :n], startw[:, :], xc[:, :n],
                                     start=True, stop=True)
                    nc.scalar.activation(mega[:, R0 + MARGIN + c0: R0 + MARGIN + c0 + n],
                                         pt[:, :n], AF.Identity,
                                         bias=startb[:, 0:1], scale=s_r)

            # ---- layer stack ----
            with tc.tile_pool(name="ps", bufs=2, space="PSUM") as ps:
                for l in range(NLAYERS):
                    offL, offR = offs[l]
                    d = offL + offR
                    rcur = R0 if l % 2 == 0 else R1
                    rnxt = R1 if l % 2 == 0 else R0
                    c_lo, c_hi = P[l]
                    for ci, c0 in enumerate(range(c_lo, c_hi, CH)):
                        n = min(CH, W - c0)
                        base = MARGIN + c0
                        fps = ps.tile([128, CH], F32, tag="f")
                        gps = ps.tile([128, CH], F32, tag="g")
                        rhs = dr_rhs(rcur + base - offL, d, n)
                        nc.tensor.matmul(fps[:, :n], w3(fgw, 2 * l + 0), rhs,
                                         start=True, stop=True, perf_mode=DR)
                        nc.tensor.matmul(gps[:, :n], w3(fgw, 2 * l + 1), rhs,
                                         start=True, stop=True, perf_mode=DR)
                        # gsb = g^ + bg'   (bf16; engine alternates for balance)
                        gsb = cnk.tile([128, CH], BF16, tag="gsb")
                        if ci % 2 == 0:
                            nc.scalar.activation(gsb[:, :n], gps[:, :n], AF.Identity,
                                                 bias=bgc[:, l:l + 1])
                        else:
                            nc.vector.tensor_scalar(gsb[:, :n], gps[:, :n],
                                                    bgc[:, l:l + 1], 0.0,
                                                    op0=A.add, op1=A.add)
                        # p~ = (f^ + bf') * gsb   -> fp8 (scale S_p, a==1)
                        po = p_off(l, c0)
                        nc.vector.scalar_tensor_tensor(
                            mega[:, po: po + n], fps[:, :n], bfc[:, l:l + 1],
                            gsb[:, :n], op0=A.add, op1=A.mult)
                        # res psum: diag @ r~ + Wr' @ p~ (K-tile order by
                        # address so the dim-1 stride stays positive <32768)
                        rps = ps.tile([128, CH], F32, tag="r")
                        ro = rcur + base
                        if l % 2 == 0:
                            rhs_ri = dr_rhs(po, ro - po, n)
                        else:
                            rhs_ri = dr_rhs(ro, po - ro, n)
                        nc.tensor.matmul(rps[:, :n], w3(resw, l), rhs_ri,
                                         start=True, stop=True, perf_mode=DR)
                        nc.scalar.activation(mega[:, rnxt + base: rnxt + base + n],
                                             rps[:, :n], AF.Identity,
                                             bias=brc[:, l:l + 1], scale=rinv_imm[l])
                        # skip quad: at l%4==3, window chunks only
                        if l % 4 == 3 and HALO_L <= c0 < HALO_L + L_CORE:
                            q = l // 4
                            oc = c0 - HALO_L
                            sps = ps.tile([128, CH], F32, tag="sk")
                            rhs_q1 = dr_rhs(RING[0] + oc, RING[2] - RING[0], n)
                            rhs_q2 = dr_rhs(RING[1] + oc, RING[3] - RING[1], n)
                            nc.tensor.matmul(sps[:, :n], w3(skw, 2 * q + 0), rhs_q1,
                                             start=True, stop=False, perf_mode=DR)
                            nc.tensor.matmul(sps[:, :n], w3(skw, 2 * q + 1), rhs_q2,
                                             start=False, stop=True, perf_mode=DR)
                            nc.vector.scalar_tensor_tensor(
                                skip[:, oc: oc + n], sps[:, :n], qinv_imm[q],
                                skip[:, oc: oc + n], op0=A.mult, op1=A.add)

            # ---- head (bf16) ----
            with tc.tile_pool(name="psf", bufs=2, space="PSUM") as psf:
                for c in range(NFIN):
                    c0 = c * CH
                    rl = cnk.tile([128, CH], BF16, tag="rl")
                    nc.scalar.activation(rl[:, :], skip[:, c0:c0 + CH], AF.Relu,
                                         bias=bst[:, 0:1])
                    for g in range(4):
                        o1sb = []
                        for h in range(2):
                            o1ps = psf.tile([128, CH], F32, tag=f"o1_{h}")
                            nc.tensor.matmul(
                                o1ps[:, :],
                                c1w[:, (2 * g + h) * 128:(2 * g + h) * 128 + 128],
                                rl[:, :], start=True, stop=True)
                            t = cnk.tile([128, CH], BF16, tag=f"o1sb_{h}")
                            nc.vector.tensor_scalar(t[:, :], o1ps[:, :],
                                                    b1w[:, h:h + 1], 0.0,
                                                    op0=A.add, op1=A.max)
                            o1sb.append(t)
                        for h2 in range(2):
                            o2ps = psf.tile([128, CH], F32, tag=f"o2_{h2}")
                            for h in range(2):
                                j = 2 * h + h2
                                nc.tensor.matmul(o2ps[:, :],
                                                 c2w[:, j * 128:(j + 1) * 128],
                                                 o1sb[h][:, :],
                                                 start=(h == 0), stop=(h == 1))
                            o2sb = cnk.tile([128, CH], F32, tag="o2sb")
                            nc.scalar.activation(o2sb[:, :], o2ps[:, :],
                                                 AF.Identity, bias=b2w[:, h2:h2 + 1])
                            nc.sync.dma_start(
                                out_d[g, 128 * h2:128 * (h2 + 1), c * CH:(c + 1) * CH],
                                o2sb[:, :])
    nc.compile()
    _NC_CACHE[key] = nc
    return nc


def _calibrate(inputs):
    """Probe on a strip for r/p maxima (exact f32)."""
    x = inputs['x'][:, 0, :4096].astype(np.float32)
    r = (inputs['w_start'][:, 0, 0][None, :, None] * x[:, None, :]
         + inputs['b_start'][None, :, None])
    r_max = np.abs(r).max()
    p_maxes = []
    S = r.shape[2]
    for blk in range(2):
        for i in range(10):
            d = 2 ** i
            offL, offR = (1, 0) if i == 0 else (d // 2, d // 2)
            rp = np.pad(r, ((0, 0), (0, 0), (offL, offR)))

            def conv(Wm):
                return (np.einsum('oc,bct->bot', Wm[:, :, 0], rp[:, :, 0:S]) +
                        np.einsum('oc,bct->bot', Wm[:, :, 1],
                                  rp[:, :, offL + offR:offL + offR + S]))
            f = conv(inputs['filt_w'][blk, i]) + inputs['filt_b'][blk, i][None, :, None]
            g = conv(inputs['gate_w'][blk, i]) + inputs['gate_b'][blk, i][None, :, None]
            p = f * g
            p_maxes.append(float(np.abs(p).max()))
            r = r + np.einsum('oc,bct->bot', inputs['res_w'][blk, i][:, :, 0], p) \
                + inputs['res_b'][blk, i][None, :, None]
            r_max = max(r_max, float(np.abs(r).max()))
    return r_max, p_maxes


def _host_arrays(inputs):
    r_max, p_maxes = _calibrate(inputs)
    S_r = _pow2(r_max * 2.0)

    fgw = np.zeros((128, NLAYERS * 2, 256), np.float32)
    resw = np.zeros((128, NLAYERS, 256), np.float32)
    skw = np.zeros((128, NQUAD * 2, 256), np.float32)
    bfc = np.zeros((128, NLAYERS), np.float32)
    bgc = np.zeros((128, NLAYERS), np.float32)
    brc = np.zeros((128, NLAYERS), np.float32)
    rinv_imm, qinv_imm = [], []
    S_p = []

    def bd(Wm):
        out = np.zeros((128, 128), np.float32)
        for g in range(4):
            out[32 * g:32 * g + 32, 32 * g:32 * g + 32] = Wm.T
        return out

    for l in range(NLAYERS):
        blk, i = divmod(l, 10)
        Wf = inputs['filt_w'][blk, i].astype(np.float32)
        Wg = inputs['gate_w'][blk, i].astype(np.float32)
        bf = inputs['filt_b'][blk, i].astype(np.float32)
        bg = inputs['gate_b'][blk, i].astype(np.float32)
        Wr = inputs['res_w'][blk, i][:, :, 0].astype(np.float32)
        br = inputs['res_b'][blk, i].astype(np.float32)
        # a == 1: S_wf*S_wg = S_p/S_r^2, centered by weight-magnitude ratio
        sp = _pow2(p_maxes[l] * 2.0)
        S_p.append(sp)
        S_prod = sp / (S_r * S_r)
        ratio = np.abs(Wg).max() / np.abs(Wf).max()
        S_wf = float(2.0 ** np.round(0.5 * np.log2(S_prod * ratio)))
        S_wg = S_prod / S_wf
        for tap in range(2):
            fgw[:, 2 * l + 0, 128 * tap:128 * tap + 128] = _q8(bd(Wf[:, :, tap] * S_wf))
            fgw[:, 2 * l + 1, 128 * tap:128 * tap + 128] = _q8(bd(Wg[:, :, tap] * S_wg))
        bfc[:, l] = np.tile(bf, 4) * S_wf * S_r
        bgc[:, l] = np.tile(bg, 4) * S_wg * S_r
        diag = _pow2(np.abs(Wr).max() * S_r / sp, target=60.0)
        diag = float(min(max(diag, 2.0 ** -9), 128.0))
        S_res = diag * S_r
        if l % 2 == 0:   # device rhs: K0 = p~, K1 = r~
            resw[:, l, 0:128] = _q8(bd(Wr * S_res / sp))
            resw[:, l, 128:256] = np.eye(128, dtype=np.float32) * diag
        else:            # K0 = r~, K1 = p~
            resw[:, l, 0:128] = np.eye(128, dtype=np.float32) * diag
            resw[:, l, 128:256] = _q8(bd(Wr * S_res / sp))
        rinv_imm.append(float(1.0 / diag))
        brc[:, l] = np.tile(br, 4) * S_r

    bs_tot = np.zeros((32,), np.float32)
    for l in range(NLAYERS):
        blk, i = divmod(l, 10)
        bs_tot += inputs['skip_b'][blk, i].astype(np.float32)
    for q in range(NQUAD):
        Wq = [inputs['skip_w'][(4 * q + j) // 10, (4 * q + j) % 10][:, :, 0].astype(np.float32)
              for j in range(4)]
        m = max(np.abs(Wq[j]).max() / S_p[4 * q + j] for j in range(4))
        Q = _pow2(m, target=60.0)
        # DR1 K-tiles = layers (4q, 4q+2); DR2 = (4q+1, 4q+3)
        for slot, j in [(0, 0), (1, 2), (2, 1), (3, 3)]:
            skw[:, 2 * q + slot // 2, 128 * (slot % 2):128 * (slot % 2) + 128] = \
                _q8(bd(Wq[j] * Q / S_p[4 * q + j]))
        qinv_imm.append(float(1.0 / Q))

    startw = np.zeros((4, 128), np.float32)
    for g in range(4):
        startw[g, 32 * g:32 * g + 32] = inputs['w_start'][:, 0, 0]
    startb = (np.tile(inputs['b_start'], 4) * S_r).reshape(128, 1).astype(np.float32)
    bst = np.tile(bs_tot, 4).reshape(128, 1).astype(np.float32)

    c1w = np.zeros((4, 128, 256), np.float32)
    for g in range(4):
        for h in range(2):
            c1w[g, 32 * g:32 * g + 32, 128 * h:128 * h + 128] = \
                inputs['w_end1'][128 * h:128 * h + 128, :, 0].T
    c1w_sb = np.ascontiguousarray(c1w.transpose(1, 0, 2).reshape(128, 1024))
    b1w = np.stack([inputs['b_end1'][0:128], inputs['b_end1'][128:256]],
                   axis=1).astype(np.float32)
    c2w = np.zeros((128, 512), np.float32)
    for h in range(2):
        for h2 in range(2):
            c2w[:, (2 * h + h2) * 128:(2 * h + h2) * 128 + 128] = \
                inputs['w_end2'][128 * h2:128 * h2 + 128, 128 * h:128 * h + 128, 0].T
    b2w = np.stack([inputs['b_end2'][0:128], inputs['b_end2'][128:256]],
                   axis=1).astype(np.float32)

    arrays = dict(fgw=fgw.reshape(128, -1), resw=resw.reshape(128, -1),
                  skw=skw.reshape(128, -1), bfc=bfc, bgc=bgc, brc=brc,
                  bst=bst, startw=startw, startb=startb,
                  c1w=c1w_sb, b1w=b1w, c2w=c2w, b2w=b2w)
    cfg = dict(rinv_imm=tuple(rinv_imm), qinv_imm=tuple(qinv_imm),
               s_r=float(S_r))
    return arrays, cfg


def _np_reference_strip(inputs, x_strip):
    S = x_strip.shape[1]

    def layer_conv(r, Wm, b, offL, offR):
        rp = np.pad(r, ((0, 0), (0, 0), (offL, offR)))
        return (np.einsum('oc,bct->bot', Wm[:, :, 0], rp[:, :, 0:S]) +
                np.einsum('oc,bct->bot', Wm[:, :, 1],
                          rp[:, :, offL + offR:offL + offR + S]) +
                b[None, :, None])

    r = (inputs['w_start'][:, 0, 0][None, :, None] * x_strip[:, None, :] +
         inputs['b_start'][None, :, None])
    skip_total = np.zeros_like(r)
    for blk in range(2):
        skip = np.zeros_like(r)
        for i in range(10):
            d = 2 ** i
            offL, offR = (1, 0) if i == 0 else (d // 2, d // 2)
            f = layer_conv(r, inputs['filt_w'][blk, i], inputs['filt_b'][blk, i], offL, offR)
            g = layer_conv(r, inputs['gate_w'][blk, i], inputs['gate_b'][blk, i], offL, offR)
            lo = f * g
            skip = skip + np.einsum('oc,bct->bot', inputs['skip_w'][blk, i][:, :, 0], lo) \
                + inputs['skip_b'][blk, i][None, :, None]
            r = r + np.einsum('oc,bct->bot', inputs['res_w'][blk, i][:, :, 0], lo) \
                + inputs['res_b'][blk, i][None, :, None]
        skip_total = skip_total + skip
    out = np.maximum(skip_total, 0)
    out = np.maximum(np.einsum('oc,bct->bot', inputs['w_end1'][:, :, 0], out) +
                     inputs['b_end1'][None, :, None], 0)
    return (np.einsum('oc,bct->bot', inputs['w_end2'][:, :, 0], out) +
            inputs['b_end2'][None, :, None])


def run(trace=False, **inputs):
    from concourse.bass_utils import run_bass_kernel_spmd
    inputs = {k: np.ascontiguousarray(np.asarray(v, np.float32)) for k, v in inputs.items()}
    shared, cfg = _host_arrays(inputs)
    nc = _build_nc(cfg)
    x = inputs['x']
    in_maps = []
    for core in range(N_CORES):
        s = core * L_CORE
        xw = np.zeros((4, WBUF), np.float32)
        lo_g, hi_g = s - HALO_L, s + L_CORE + HALO_R
        lo_c, hi_c = max(lo_g, 0), min(hi_g, L)
        xw[:, MARGIN + lo_c - lo_g: MARGIN + lo_c - lo_g + (hi_c - lo_c)] = \
            x[:, 0, lo_c:hi_c]
        m = {"xw": xw}
        m.update(shared)
        in_maps.append(m)
    res = run_bass_kernel_spmd(nc, in_maps, core_ids=list(range(N_CORES)),
                               trace=trace)
    out = np.zeros((B, 256, L), np.float32)
    for core in range(N_CORES):
        out[:, :, core * L_CORE:(core + 1) * L_CORE] = res.results[core]["out"]
    STRIP = 2048
    left = _np_reference_strip(inputs, x[:, 0, :STRIP])
    out[:, :, :HALO_L] = left[:, :, :HALO_L]
    right = _np_reference_strip(inputs, x[:, 0, L - STRIP:])
    out[:, :, L - HALO_L:] = right[:, :, STRIP - HALO_L:]
    return out, res


def kernel(**inputs) -> np.ndarray:
    out, _ = run(trace=False, **inputs)
    return out
